# revision 11
# baseline (speedup 1.0000x reference)
"""Trainium2 Bass kernel for nn_CustomModel_12953621365157 (gnn_message_passing).

Strategy
--------
Data-parallel over the batch axis: 8 cores, 512 batch columns each.

Per layer the reference does gather(edge_src) -> 0.5-weight-with-|g|==1-quirk
-> segment_sum(edge_dst) -> per-node activation.  Because the quirk depends
only on the gathered *value*, it folds into the source node:

    v_adj[n] = v[n] + c*(v[n]==1) - c*(v[n]==-1),   c = (1-w)/w
    summed   = w * (A_l @ v_adj)                    A_l[p,n] = #edges n->p

so each layer is a dense [P x N_l] x [N_l x B_s] matmul on the PE.  A_l is
built host-side from the edge lists and stored bf16 (counts <= 4, exact);
the moving operand V stays float32r so the PE still runs 1 col/cycle.

Pipeline layout (v2):
- Matmuls are emitted k-outer (source-tile outer, dst-chunk inner) in two
  phases per layer: OLD sources (available >= 1 layer ago) then the 4 NEW
  source tiles (previous layer's output).  The PE only ever waits for the
  previous layer's V tiles at the last 16 matmuls of a layer, and DMA of A
  streams one tile ahead of the PE at the start.
- Nodes of each layer are pre-sorted by activation id with sin/cos FIRST
  (their DVE range-reduction chain starts as soon as psum chunk 0 lands)
  and step LAST (so the chunk-3 quirk custom op keeps partition base 0).
- Cheap elementwise post-ops (step, linear, invert, sigmoid-affine, abs,
  relu) run on the idle Pool engine; DVE keeps only the trig chain and the
  quirk fold; ACT keeps table funcs (sin/cos/tanh/square/exp).
"""

import numpy as np

N_IN = 512
P = 512
L = 4
E = 32768
B = 4096
N_CORES = 8
N_ACT = 10
BS = B // N_CORES  # 512 batch columns per core

# activation ids (order matches reference activations_dict)
LINEAR, STEP, SIN, COS, GAUSS, TANH, SIGMOID, ABS, INVERT, RELU = range(10)

TILE_BASE = [0, 4, 12, 24]  # first A-tile index of each layer; 40 tiles total
N_A_TILES = 40

A_DTYPE = "f32r"  # "f32r" | "bf16" | "fp8e4"


def _preprocess(x, w, edge_src, edge_dst, act_ids):
    """Host-side: node sort per layer, dense A build, input quirk fold."""
    c = (1.0 - w) / w

    perms = []      # perms[l][p_sorted] = orig node j
    inv_perms = []  # inv_perms[l][orig j] = p_sorted
    segs = []       # segs[l][m] = list of (func_id, lo, hi) within chunk m
    for l in range(L):
        ids = np.asarray(act_ids[l])
        # Custom layout (bottom -> top of the 512 partitions):
        #   [head filler][SIN][COS][rest of filler][STEP]
        # * sin/cos end exactly at partition 128 (top of chunk 0): the
        #   descending per-chunk emission then puts the trig ACT instrs
        #   first in the layer -> act tables group [trig][exp], 2 loads
        #   per layer, while the DVE range-reduction chain still starts
        #   as soon as psum chunk 0 is finished.
        # * step last: the chunk-3 quirk custom op keeps partition base 0.
        by_f = {f: list(np.nonzero(ids == f)[0]) for f in range(N_ACT)}
        trig = by_f[SIN] + by_f[COS]
        filler = []
        for f in (GAUSS, TANH, ABS, SIGMOID, RELU, LINEAR, INVERT):
            filler += by_f[f]
        head = max(0, 128 - len(trig))
        perm = np.array(filler[:head] + trig + filler[head:] + by_f[STEP],
                        np.int64)
        assert len(perm) == P
        inv = np.empty(P, np.int64)
        inv[perm] = np.arange(P)
        perms.append(perm)
        inv_perms.append(inv)
        ids_sorted = ids[perm]
        layer_segs = []
        for m in range(4):
            chunk = ids_sorted[m * 128:(m + 1) * 128]
            runs = []
            lo = 0
            for i in range(1, 129):
                if i == 128 or chunk[i] != chunk[lo]:
                    runs.append((int(chunk[lo]), lo, i))
                    lo = i
            layer_segs.append(runs)
        segs.append(layer_segs)

    inv_stack = np.stack(inv_perms)  # [L, P]

    a_pack = np.zeros((N_A_TILES * 128, P), np.float32)
    for l in range(L):
        src = np.asarray(edge_src[l]).astype(np.int64)
        dst = np.asarray(edge_dst[l]).astype(np.int64)
        g = src.copy()
        m = g >= N_IN
        lp = (g[m] - N_IN) // P
        j = (g[m] - N_IN) % P
        g[m] = N_IN + lp * P + inv_stack[lp, j]
        d = inv_perms[l][dst]
        np.add.at(a_pack, (TILE_BASE[l] * 128 + g, d), 1.0)

    xa = x.astype(np.float32)
    if c != 0.0:
        xa = xa + c * (xa == 1.0) - c * (xa == -1.0)
    xin = -xa  # V tiles hold -v_adj
    return a_pack, xin.astype(np.float32), perms, segs


_QUIRK_OP = None


def _get_quirk_fold_op():
    """Custom single-uop DVE op: out = (in==-1) - ((in==1) + in) = -v_adj.

    NOTE: custom DVE ops only work on APs with partition base 0 (silently
    no-op otherwise); all uses here keep base 0.
    """
    global _QUIRK_OP
    if _QUIRK_OP is not None:
        return _QUIRK_OP
    import concourse.dve_ops as dve_ops
    from concourse.dve_spec import (Spec, Src0, C0, C1, Bin, AluOp, lower,
                                    _has_src1)
    from concourse.dve_uop import DveOpSpec

    def eq(a, b):
        return Bin(AluOp.IS_EQ, a, b)

    spec = Spec(
        body=eq(Src0, C1) - (eq(Src0, C0) + Src0),
        reference=lambda in0, s0, s1, imm2: (
            (in0 == s1).astype(np.float32)
            - ((in0 == s0).astype(np.float32) + in0)),
    )
    name = "QUIRK_FOLD_ANT"
    if name not in dve_ops._SUB_OPCODE_FOR_NAME:
        row = max(dve_ops._SUB_OPCODE_FOR_NAME.values()) + 1
        assert row < 0x20
        dve_ops._SUB_OPCODE_FOR_NAME[name] = row
    opcode = dve_ops._SUB_OPCODE_FOR_NAME[name]
    shas = {}
    for ver in ("v3", "v4"):
        u = lower(spec, ver=ver)
        shas[ver] = DveOpSpec(name=name, opcode=opcode, uops=u,
                              rd1_en=_has_src1(spec)).sha(ver)
    op = dve_ops.DveOp(name, spec, subdim=False, uops_sha=shas)
    if all(o.name != name for o in dve_ops.OPS):
        dve_ops.OPS.append(op)
    dve_ops.CUSTOM_DVE_SPECS[name] = spec
    _QUIRK_OP = op
    return op


def _build_program(segs, w):
    import concourse.bass as bass
    import concourse.bacc as bacc
    import concourse.mybir as mybir
    import concourse.tile as tile
    from concourse.tile_rust import add_dep_helper

    quirk_op = _get_quirk_fold_op()

    dt = mybir.dt
    Act = mybir.ActivationFunctionType
    Alu = mybir.AluOpType
    W = float(w)

    a_dt = {"f32r": dt.float32r, "bf16": dt.bfloat16,
            "fp8e4": dt.float8e4}[A_DTYPE]

    # Cody-Waite split of 2*pi: c1/c2 short so k*c1, k*c2 are exact for
    # |k| < 2^12; c3 carries the remainder.
    def _trunc(x, bits):
        u = np.float32(x).view(np.uint32)
        mask = np.uint32(0xFFFFFFFF) << np.uint32(23 - bits)
        return float((u & mask).view(np.float32))

    TWO_PI = 2.0 * np.pi
    CW1 = _trunc(TWO_PI, 7)                    # 6.28125, exact
    CW2 = _trunc(TWO_PI - CW1, 12)
    CW3 = float(np.float32(TWO_PI - CW1 - CW2))
    INV_2PI = float(np.float32(1.0 / TWO_PI))
    PI_F = float(np.float32(np.pi))
    HALF_PI = float(np.float32(np.pi / 2))
    TWO_PI_F = float(np.float32(TWO_PI))
    c = (1.0 - W) / W
    fast_chain = (W == 0.5)

    nc = bacc.Bacc("TRN2", target_bir_lowering=False, debug=False,
                   num_devices=N_CORES)
    xin = nc.dram_tensor("xin", [N_IN, BS], dt.float32r,
                         kind="ExternalInput").ap()
    a_d = nc.dram_tensor("amat", [N_A_TILES * 128, P], a_dt,
                         kind="ExternalInput").ap()
    out_d = nc.dram_tensor("out", [P, BS], dt.float32,
                           kind="ExternalOutput").ap()

    with tile.TileContext(nc) as tc:
        with tc.tile_pool(name="Ap", bufs=1) as apool, \
             tc.tile_pool(name="Vp", bufs=1) as vpool, \
             tc.tile_pool(name="raw", bufs=6) as rpool, \
             tc.tile_pool(name="ps", bufs=8, space="PSUM") as ppool:

            # input node values (already quirk-folded & negated on host),
            # interleaved with layer-0 A tiles so the first matmuls can
            # start after ~0.5 MB of DMA.
            V = []
            A = {}
            for k in range(4):
                vt = vpool.tile([128, BS], dt.float32r, name=f"v{k}")
                nc.sync.dma_start(vt[:], xin[k * 128:(k + 1) * 128, :])
                V.append(vt)
                at = apool.tile([128, P], a_dt, name=f"a0_{k}")
                r0 = (TILE_BASE[0] + k) * 128
                nc.sync.dma_start(at[:], a_d[r0:r0 + 128, :])
                A[(0, k)] = at
            for l in range(1, L):
                for k in range(4 + 4 * l):
                    at = apool.tile([128, P], a_dt, name=f"a{l}_{k}")
                    r0 = (TILE_BASE[l] + k) * 128
                    nc.sync.dma_start(at[:], a_d[r0:r0 + 128, :])
                    A[(l, k)] = at

            def _pieces(lo, hi):
                p = (lo // 32) * 32
                out = []
                while p < hi:
                    end = min(hi, 64) if p == 32 else hi
                    out.append((p, end))
                    p = end
                return out

            act_chain = []

            for l in range(L):
                nk = 4 + 4 * l
                # --- matmuls: k-outer so each A tile / V tile unblocks a
                # block of 4 chunk-matmuls as soon as it lands.
                psums = [ppool.tile([128, BS], dt.float32, name="ps")
                         for _ in range(4)]
                # OLD sources (k-outer: each newly-available A/V tile
                # unblocks 4 matmuls; no dependency on the previous layer),
                # then the last 4 source tiles m-outer so psum chunks
                # complete staggered (chunk 0 first, 4 matmuls apart).
                for k in range(nk - 4):
                    ak = A[(l, k)]
                    for m in range(4):
                        nc.tensor.matmul(
                            psums[m][:], ak[:, m * 128:(m + 1) * 128],
                            V[k][:], start=(k == 0), stop=False)
                for m in range(4):
                    for k in range(nk - 4, nk):
                        ak = A[(l, k)]
                        nc.tensor.matmul(
                            psums[m][:], ak[:, m * 128:(m + 1) * 128],
                            V[k][:], start=(k == 0), stop=(k == nk - 1))

                # --- trig prep (chunk 0 holds all sin/cos nodes): full-tile
                # Cody-Waite range reduction on the DVE.
                has_sin = any(f == SIN for f, _, _ in segs[l][0])
                has_cos = any(f == COS for f, _, _ in segs[l][0])
                wsin = wcos = None
                if has_sin or has_cos:
                    ps0 = psums[0]
                    sq = rpool.tile([128, BS], dt.float32, name="sq", bufs=2)
                    si = rpool.tile([128, BS], dt.int32, name="si", bufs=2)
                    sz = rpool.tile([128, BS], dt.float32, name="sz", bufs=2)
                    sr = rpool.tile([128, BS], dt.float32, name="sr", bufs=2)
                    nc.vector.tensor_scalar(si[:], ps0[:], -W * INV_2PI,
                                            None, Alu.mult)  # i32 out: k
                    nc.vector.tensor_copy(sq[:], si[:])   # i32 -> f32 (= k)
                    nc.vector.tensor_scalar(sz[:], ps0[:], -W, None, Alu.mult)
                    nc.vector.cody_waite_cascade(sr[:], sz[:], sq[:],
                                                 CW1, CW2, CW3)
                    nc.vector.add_range_wrap(sz[:], sr[:], 0.0, PI_F,
                                             TWO_PI_F)
                    wsin = sz
                    if has_cos:
                        nc.vector.add_range_wrap(sq[:], sr[:], HALF_PI, PI_F,
                                                 TWO_PI_F)
                        wcos = sq

                # --- per-chunk activations, chunks ascending (chunk 0 first
                # = V-tile order consumed by the next layer).  Within a
                # chunk, segments are emitted in DESCENDING partition order:
                # pieces are extended down to 32-aligned starts, so the true
                # owner of every overlap region writes last.  ACT instrs are
                # chained so table loads group: [sin,cos] (trig set) then
                # [square/exp/tanh] (exp set), 2 loads per layer.
                for m in range(4):
                    ps = psums[m]
                    vraw = rpool.tile([128, BS], dt.float32, name="vraw")
                    tmp = rpool.tile([128, BS], dt.float32, name="tmp")
                    last = (l == L - 1)
                    vt = None
                    step_lo = None
                    if not last:
                        vt = vpool.tile([128, BS], dt.float32r,
                                        name=f"v{4 + 4 * l + m}")

                    for fid, slo, shi in reversed(segs[l][m]):
                      for lo, hi in _pieces(slo, shi):
                        s = np.s_[lo:hi, :]
                        if fid == GAUSS:
                            act_chain.append(nc.scalar.activation(
                                tmp[s], ps[s], Act.Square, scale=-W))
                            act_chain.append(nc.scalar.activation(
                                vraw[s], tmp[s], Act.Exp, scale=-1.0))
                        elif fid in (SIN, COS):
                            src_t = wsin if fid == SIN else wcos
                            act_chain.append(nc.scalar.activation(
                                vraw[s], src_t[s], Act.Sin, scale=1.0))
                        elif fid == TANH:
                            act_chain.append(nc.scalar.activation(
                                vraw[s], ps[s], Act.Tanh, scale=-W))
                        elif fid == SIGMOID:
                            act_chain.append(nc.scalar.activation(
                                tmp[s], ps[s], Act.Tanh, scale=-W / 2))
                            nc.gpsimd.tensor_scalar(vraw[s], tmp[s], 0.5, 0.5,
                                                    Alu.mult, Alu.add)
                        elif fid == STEP:
                            # step(S_true) = +1 iff S_psum <= 0
                            nc.vector.tensor_scalar(tmp[s], ps[s], 0.0, None,
                                                    Alu.is_le)
                            if last:
                                nc.gpsimd.tensor_scalar(
                                    vraw[s], tmp[s], 2.0, 1.0,
                                    Alu.mult, Alu.subtract)
                            else:
                                # write -v_adj = -2*step directly; the quirk
                                # op below skips [step_lo, 128).
                                step_lo = slo
                                nc.gpsimd.tensor_scalar(
                                    vt[s], tmp[s],
                                    -4.0, 2.0, Alu.mult, Alu.add)
                        elif fid == ABS:
                            act_chain.append(nc.scalar.activation(
                                vraw[s], ps[s], Act.Abs, scale=-W))
                        elif fid == INVERT:
                            nc.vector.tensor_scalar(vraw[s], ps[s], W, None,
                                                    Alu.mult)
                        elif fid == LINEAR:
                            nc.vector.tensor_scalar(vraw[s], ps[s], -W, None,
                                                    Alu.mult)
                        elif fid == RELU:
                            act_chain.append(nc.scalar.activation(
                                vraw[s], ps[s], Act.Relu, scale=-W))
                        else:
                            raise ValueError(fid)

                    if last:
                        nc.sync.dma_start(out_d[m * 128:(m + 1) * 128, :],
                                          vraw[:])
                    else:
                        qhi = 128 if step_lo is None else step_lo
                        qs = np.s_[0:qhi, :]
                        if fast_chain:
                            nc.vector._custom_dve(
                                quirk_op, out=vt[qs], in0=vraw[qs],
                                s0=1.0, s1=-1.0)
                        else:
                            m1c = rpool.tile([128, BS], dt.float32,
                                             name="m1c")
                            nc.vector.tensor_scalar(m1c[qs], vraw[qs], 1.0, c,
                                                    Alu.is_equal, Alu.mult)
                            nc.vector.tensor_tensor(tmp[qs], m1c[qs],
                                                    vraw[qs], Alu.add)
                            nc.vector.tensor_scalar(m1c[qs], vraw[qs], -1.0,
                                                    c, Alu.is_equal, Alu.mult)
                            nc.vector.tensor_tensor(
                                vt[qs], m1c[qs], tmp[qs],
                                Alu.subtract)
                        V.append(vt)

            # pin ACT table order: trig group then exp group, per layer.
            for a, b in zip(act_chain, act_chain[1:]):
                add_dep_helper(b.ins, a.ins, sync=False,
                               reason="act table order")
    nc.compile()
    return nc


_CACHE = {}


def _get_program(segs_key, segs, w):
    key = (segs_key, float(w))
    if key not in _CACHE:
        _CACHE[key] = _build_program(segs, w)
    return _CACHE[key]


def kernel(x, shared_weight, edge_src, edge_dst, act_ids):
    import concourse.mybir as mybir
    from concourse.bass_utils import run_bass_kernel_spmd

    w = float(np.asarray(shared_weight))
    assert w != 0.0
    a_pack, xin, perms, segs = _preprocess(
        np.asarray(x), w, np.asarray(edge_src), np.asarray(edge_dst),
        np.asarray(act_ids))

    segs_key = tuple(tuple(tuple(r) for r in lm) for lseg in segs for lm in lseg)
    nc = _get_program(segs_key, segs, w)

    dt = mybir.dt
    a_np = {"f32r": np.float32, "bf16": mybir.dt.np(dt.bfloat16),
            "fp8e4": mybir.dt.np(dt.float8e4)}[A_DTYPE]
    a_cast = a_pack.astype(a_np)

    in_maps = [
        {"xin": np.ascontiguousarray(xin[:, cid * BS:(cid + 1) * BS]),
         "amat": a_cast}
        for cid in range(N_CORES)
    ]
    res = run_bass_kernel_spmd(nc, in_maps, core_ids=list(range(N_CORES)))
    out_sorted = np.concatenate([res.results[cid]["out"]
                                 for cid in range(N_CORES)], axis=1)
    out = np.empty_like(out_sorted)
    out[perms[L - 1]] = out_sorted
    return out.astype(np.float32)


# revision 13
# speedup vs baseline: 1.1364x; 1.1364x over previous
"""Trainium2 Bass kernel for nn_CustomModel_12953621365157 (gnn_message_passing).

Strategy
--------
Data-parallel over the batch axis: 8 cores, 512 batch columns each.

Per layer the reference does gather(edge_src) -> 0.5-weight-with-|g|==1-quirk
-> segment_sum(edge_dst) -> per-node activation.  Because the quirk depends
only on the gathered *value*, it folds into the source node:

    v_adj[n] = v[n] + c*(v[n]==1) - c*(v[n]==-1),   c = (1-w)/w
    summed   = w * (A_l @ v_adj)                    A_l[p,n] = #edges n->p

so each layer is a dense [P x N_l] x [N_l x B_s] matmul on the PE.  A_l is
built host-side from the edge lists and stored bf16 (counts <= 4, exact);
the moving operand V stays float32r so the PE still runs 1 col/cycle.

Pipeline layout (v2):
- Matmuls are emitted k-outer (source-tile outer, dst-chunk inner) in two
  phases per layer: OLD sources (available >= 1 layer ago) then the 4 NEW
  source tiles (previous layer's output).  The PE only ever waits for the
  previous layer's V tiles at the last 16 matmuls of a layer, and DMA of A
  streams one tile ahead of the PE at the start.
- Nodes of each layer are pre-sorted by activation id with sin/cos FIRST
  (their DVE range-reduction chain starts as soon as psum chunk 0 lands)
  and step LAST (so the chunk-3 quirk custom op keeps partition base 0).
- Cheap elementwise post-ops (step, linear, invert, sigmoid-affine, abs,
  relu) run on the idle Pool engine; DVE keeps only the trig chain and the
  quirk fold; ACT keeps table funcs (sin/cos/tanh/square/exp).
"""

import numpy as np

N_IN = 512
P = 512
L = 4
E = 32768
B = 4096
N_CORES = 8
N_ACT = 10
BS = B // N_CORES  # 512 batch columns per core

# activation ids (order matches reference activations_dict)
LINEAR, STEP, SIN, COS, GAUSS, TANH, SIGMOID, ABS, INVERT, RELU = range(10)

TILE_BASE = [0, 4, 12, 24]  # first A-tile index of each layer; 40 tiles total
N_A_TILES = 40

A_DTYPE = "f32r"  # "f32r" | "bf16" | "fp8e4"


def _preprocess(x, w, edge_src, edge_dst, act_ids):
    """Host-side: node sort per layer, dense A build, input quirk fold."""
    c = (1.0 - w) / w

    perms = []      # perms[l][p_sorted] = orig node j
    inv_perms = []  # inv_perms[l][orig j] = p_sorted
    segs = []       # segs[l][m] = list of (func_id, lo, hi) within chunk m
    for l in range(L):
        ids = np.asarray(act_ids[l])
        # Custom layout (bottom -> top of the 512 partitions):
        #   [head filler][SIN][COS][rest of filler][STEP]
        # * sin/cos end exactly at partition 128 (top of chunk 0): the
        #   descending per-chunk emission then puts the trig ACT instrs
        #   first in the layer -> act tables group [trig][exp], 2 loads
        #   per layer, while the DVE range-reduction chain still starts
        #   as soon as psum chunk 0 is finished.
        # * step last: the chunk-3 quirk custom op keeps partition base 0.
        by_f = {f: list(np.nonzero(ids == f)[0]) for f in range(N_ACT)}
        trig = by_f[SIN] + by_f[COS]
        filler = []
        for f in (GAUSS, TANH, ABS, SIGMOID, RELU, LINEAR, INVERT):
            filler += by_f[f]
        head = (max(0, 128 - len(trig)) // 32) * 32
        perm = np.array(filler[:head] + trig + filler[head:] + by_f[STEP],
                        np.int64)
        assert len(perm) == P
        inv = np.empty(P, np.int64)
        inv[perm] = np.arange(P)
        perms.append(perm)
        inv_perms.append(inv)
        ids_sorted = ids[perm]
        layer_segs = []
        for m in range(4):
            chunk = ids_sorted[m * 128:(m + 1) * 128]
            runs = []
            lo = 0
            for i in range(1, 129):
                if i == 128 or chunk[i] != chunk[lo]:
                    runs.append((int(chunk[lo]), lo, i))
                    lo = i
            layer_segs.append(runs)
        segs.append(layer_segs)

    inv_stack = np.stack(inv_perms)  # [L, P]

    a_pack = np.zeros((N_A_TILES * 128, P), np.float32)
    for l in range(L):
        src = np.asarray(edge_src[l]).astype(np.int64)
        dst = np.asarray(edge_dst[l]).astype(np.int64)
        g = src.copy()
        m = g >= N_IN
        lp = (g[m] - N_IN) // P
        j = (g[m] - N_IN) % P
        g[m] = N_IN + lp * P + inv_stack[lp, j]
        d = inv_perms[l][dst]
        np.add.at(a_pack, (TILE_BASE[l] * 128 + g, d), 1.0)

    xa = x.astype(np.float32)
    if c != 0.0:
        xa = xa + c * (xa == 1.0) - c * (xa == -1.0)
    xin = -xa  # V tiles hold -v_adj
    return a_pack, xin.astype(np.float32), perms, segs


_QUIRK_OP = None


def _get_quirk_fold_op():
    """Custom single-uop DVE op: out = (in==-1) - ((in==1) + in) = -v_adj.

    NOTE: custom DVE ops only work on APs with partition base 0 (silently
    no-op otherwise); all uses here keep base 0.
    """
    global _QUIRK_OP
    if _QUIRK_OP is not None:
        return _QUIRK_OP
    import concourse.dve_ops as dve_ops
    from concourse.dve_spec import (Spec, Src0, C0, C1, Bin, AluOp, lower,
                                    _has_src1)
    from concourse.dve_uop import DveOpSpec

    def eq(a, b):
        return Bin(AluOp.IS_EQ, a, b)

    spec = Spec(
        body=eq(Src0, C1) - (eq(Src0, C0) + Src0),
        reference=lambda in0, s0, s1, imm2: (
            (in0 == s1).astype(np.float32)
            - ((in0 == s0).astype(np.float32) + in0)),
    )
    name = "QUIRK_FOLD_ANT"
    if name not in dve_ops._SUB_OPCODE_FOR_NAME:
        row = max(dve_ops._SUB_OPCODE_FOR_NAME.values()) + 1
        assert row < 0x20
        dve_ops._SUB_OPCODE_FOR_NAME[name] = row
    opcode = dve_ops._SUB_OPCODE_FOR_NAME[name]
    shas = {}
    for ver in ("v3", "v4"):
        u = lower(spec, ver=ver)
        shas[ver] = DveOpSpec(name=name, opcode=opcode, uops=u,
                              rd1_en=_has_src1(spec)).sha(ver)
    op = dve_ops.DveOp(name, spec, subdim=False, uops_sha=shas)
    if all(o.name != name for o in dve_ops.OPS):
        dve_ops.OPS.append(op)
    dve_ops.CUSTOM_DVE_SPECS[name] = spec
    _QUIRK_OP = op
    return op


def _build_program(segs, w):
    import concourse.bass as bass
    import concourse.bacc as bacc
    import concourse.mybir as mybir
    import concourse.tile as tile
    from concourse.tile_rust import add_dep_helper

    quirk_op = _get_quirk_fold_op()

    dt = mybir.dt
    Act = mybir.ActivationFunctionType
    Alu = mybir.AluOpType
    W = float(w)

    a_dt = {"f32r": dt.float32r, "bf16": dt.bfloat16,
            "fp8e4": dt.float8e4}[A_DTYPE]

    # Cody-Waite split of 2*pi: c1/c2 short so k*c1, k*c2 are exact for
    # |k| < 2^12; c3 carries the remainder.
    def _trunc(x, bits):
        u = np.float32(x).view(np.uint32)
        mask = np.uint32(0xFFFFFFFF) << np.uint32(23 - bits)
        return float((u & mask).view(np.float32))

    TWO_PI = 2.0 * np.pi
    CW1 = _trunc(TWO_PI, 7)                    # 6.28125, exact
    CW2 = _trunc(TWO_PI - CW1, 12)
    CW3 = float(np.float32(TWO_PI - CW1 - CW2))
    INV_2PI = float(np.float32(1.0 / TWO_PI))
    PI_F = float(np.float32(np.pi))
    HALF_PI = float(np.float32(np.pi / 2))
    TWO_PI_F = float(np.float32(TWO_PI))
    c = (1.0 - W) / W
    fast_chain = (W == 0.5)

    nc = bacc.Bacc("TRN2", target_bir_lowering=False, debug=False,
                   num_devices=N_CORES)
    xin = nc.dram_tensor("xin", [N_IN, BS], dt.float32r,
                         kind="ExternalInput").ap()
    a_d = nc.dram_tensor("amat", [N_A_TILES * 128, P], a_dt,
                         kind="ExternalInput").ap()
    out_d = nc.dram_tensor("out", [P, BS], dt.float32,
                           kind="ExternalOutput").ap()

    with tile.TileContext(nc) as tc:
        with tc.tile_pool(name="Ap", bufs=1) as apool, \
             tc.tile_pool(name="Vp", bufs=1) as vpool, \
             tc.tile_pool(name="raw", bufs=6) as rpool, \
             tc.tile_pool(name="ps", bufs=8, space="PSUM") as ppool:

            # input node values (already quirk-folded & negated on host),
            # interleaved with layer-0 A tiles so the first matmuls can
            # start after ~0.5 MB of DMA.
            V = []
            A = {}
            for k in range(4):
                vt = vpool.tile([128, BS], dt.float32r, name=f"v{k}")
                nc.sync.dma_start(vt[:], xin[k * 128:(k + 1) * 128, :])
                V.append(vt)
                at = apool.tile([128, P], a_dt, name=f"a0_{k}")
                r0 = (TILE_BASE[0] + k) * 128
                nc.sync.dma_start(at[:], a_d[r0:r0 + 128, :])
                A[(0, k)] = at
            for l in range(1, L):
                for k in range(4 + 4 * l):
                    at = apool.tile([128, P], a_dt, name=f"a{l}_{k}")
                    r0 = (TILE_BASE[l] + k) * 128
                    nc.sync.dma_start(at[:], a_d[r0:r0 + 128, :])
                    A[(l, k)] = at

            def _pieces(lo, hi):
                p = (lo // 32) * 32
                out = []
                while p < hi:
                    end = min(hi, 64) if p == 32 else hi
                    out.append((p, end))
                    p = end
                return out

            act_chain = []

            for l in range(L):
                nk = 4 + 4 * l
                # --- matmuls: k-outer so each A tile / V tile unblocks a
                # block of 4 chunk-matmuls as soon as it lands.
                psums = [ppool.tile([128, BS], dt.float32, name="ps")
                         for _ in range(4)]
                # OLD sources (k-outer: each newly-available A/V tile
                # unblocks 4 matmuls; no dependency on the previous layer),
                # then the last 4 source tiles m-outer so psum chunks
                # complete staggered (chunk 0 first, 4 matmuls apart).
                for k in range(nk - 4):
                    ak = A[(l, k)]
                    for m in range(4):
                        nc.tensor.matmul(
                            psums[m][:], ak[:, m * 128:(m + 1) * 128],
                            V[k][:], start=(k == 0), stop=False)
                korder = ([0, 1, 2, 3] if l == 0 else [1, 2, 3, 0])
                for m in range(4):
                    for i, kk in enumerate(korder):
                        k = nk - 4 + kk
                        ak = A[(l, k)]
                        nc.tensor.matmul(
                            psums[m][:], ak[:, m * 128:(m + 1) * 128],
                            V[k][:], start=(l == 0 and i == 0),
                            stop=(i == 3))

                # --- trig prep (chunk 0 holds all sin/cos nodes): full-tile
                # Cody-Waite range reduction on the DVE.
                has_sin = any(f == SIN for f, _, _ in segs[l][0])
                has_cos = any(f == COS for f, _, _ in segs[l][0])
                wsin = wcos = None
                if has_sin or has_cos:
                    ps0 = psums[0]
                    sq = rpool.tile([128, BS], dt.float32, name="sq", bufs=2)
                    si = rpool.tile([128, BS], dt.int32, name="si", bufs=2)
                    sz = rpool.tile([128, BS], dt.float32, name="sz", bufs=2)
                    sr = rpool.tile([128, BS], dt.float32, name="sr", bufs=2)
                    nc.vector.tensor_scalar(si[:], ps0[:], -W * INV_2PI,
                                            None, Alu.mult)  # i32 out: k
                    nc.vector.tensor_copy(sq[:], si[:])   # i32 -> f32 (= k)
                    nc.vector.tensor_scalar(sz[:], ps0[:], -W, None, Alu.mult)
                    nc.vector.cody_waite_cascade(sr[:], sz[:], sq[:],
                                                 CW1, CW2, CW3)
                    nc.vector.add_range_wrap(sz[:], sr[:], 0.0, PI_F,
                                             TWO_PI_F)
                    wsin = sz
                    if has_cos:
                        nc.vector.add_range_wrap(sq[:], sr[:], HALF_PI, PI_F,
                                                 TWO_PI_F)
                        wcos = sq

                # --- per-chunk activations, chunks ascending (chunk 0 first
                # = V-tile order consumed by the next layer).  Within a
                # chunk, segments are emitted in DESCENDING partition order:
                # pieces are extended down to 32-aligned starts, so the true
                # owner of every overlap region writes last.  ACT instrs are
                # chained so table loads group: [sin,cos] (trig set) then
                # [square/exp/tanh] (exp set), 2 loads per layer.
                trig_defer = []
                for m in range(4):
                    ps = psums[m]
                    vraw = rpool.tile([128, BS], dt.float32, name="vraw")
                    tmp = rpool.tile([128, BS], dt.float32, name="tmp")
                    last = (l == L - 1)
                    vt = None
                    step_lo = None
                    if not last:
                        vt = vpool.tile([128, BS], dt.float32r,
                                        name=f"v{4 + 4 * l + m}")

                    for fid, slo, shi in reversed(segs[l][m]):
                      for lo, hi in _pieces(slo, shi):
                        s = np.s_[lo:hi, :]
                        if fid == GAUSS:
                            act_chain.append(nc.scalar.activation(
                                tmp[s], ps[s], Act.Square, scale=-W))
                            act_chain.append(nc.scalar.activation(
                                vraw[s], tmp[s], Act.Exp, scale=-1.0))
                        elif fid in (SIN, COS):
                            src_t = wsin if fid == SIN else wcos
                            trig_defer.append((vraw, src_t, s))
                            continue
                        elif fid == TANH:
                            act_chain.append(nc.scalar.activation(
                                vraw[s], ps[s], Act.Tanh, scale=-W))
                        elif fid == SIGMOID:
                            act_chain.append(nc.scalar.activation(
                                tmp[s], ps[s], Act.Tanh, scale=-W / 2))
                            nc.gpsimd.tensor_scalar(vraw[s], tmp[s], 0.5, 0.5,
                                                    Alu.mult, Alu.add)
                        elif fid == STEP:
                            # step(S_true) = +1 iff S_psum <= 0
                            nc.vector.tensor_scalar(tmp[s], ps[s], 0.0, None,
                                                    Alu.is_le)
                            if last:
                                nc.gpsimd.tensor_scalar(
                                    vraw[s], tmp[s], 2.0, 1.0,
                                    Alu.mult, Alu.subtract)
                            else:
                                # write -v_adj = -2*step directly; the quirk
                                # op below skips [step_lo, 128).
                                step_lo = slo
                                nc.gpsimd.tensor_scalar(
                                    vt[s], tmp[s],
                                    -4.0, 2.0, Alu.mult, Alu.add)
                        elif fid == ABS:
                            act_chain.append(nc.scalar.activation(
                                vraw[s], ps[s], Act.Abs, scale=-W))
                        elif fid == INVERT:
                            nc.vector.tensor_scalar(vraw[s], ps[s], W, None,
                                                    Alu.mult)
                        elif fid == LINEAR:
                            nc.vector.tensor_scalar(vraw[s], ps[s], -W, None,
                                                    Alu.mult)
                        elif fid == RELU:
                            act_chain.append(nc.scalar.activation(
                                vraw[s], ps[s], Act.Relu, scale=-W))
                        else:
                            raise ValueError(fid)

                    def _finalize(m, vraw, vt, tmp, step_lo):
                        if l == L - 1:
                            nc.sync.dma_start(
                                out_d[m * 128:(m + 1) * 128, :], vraw[:])
                            return
                        qhi = 128 if step_lo is None else step_lo
                        qs = np.s_[0:qhi, :]
                        if fast_chain:
                            nc.vector._custom_dve(
                                quirk_op, out=vt[qs], in0=vraw[qs],
                                s0=1.0, s1=-1.0)
                        else:
                            m1c = rpool.tile([128, BS], dt.float32,
                                             name="m1c")
                            nc.vector.tensor_scalar(m1c[qs], vraw[qs], 1.0, c,
                                                    Alu.is_equal, Alu.mult)
                            nc.vector.tensor_tensor(tmp[qs], m1c[qs],
                                                    vraw[qs], Alu.add)
                            nc.vector.tensor_scalar(m1c[qs], vraw[qs], -1.0,
                                                    c, Alu.is_equal, Alu.mult)
                            nc.vector.tensor_tensor(
                                vt[qs], m1c[qs], tmp[qs],
                                Alu.subtract)

                    if m == 0 and trig_defer:
                        # chunk 0 finalizes after the deferred sin/cos
                        # instrs at the end of the layer's act phase.
                        defer0 = (m, vraw, vt, tmp, step_lo)
                    else:
                        _finalize(m, vraw, vt, tmp, step_lo)
                    if vt is not None:
                        V.append(vt)

                # deferred trig ACT instrs: table order [exp ...][trig] per
                # layer; ownership holds because the trig region starts on a
                # 32-aligned partition (no downward extension below it).
                for vraw_t, src_t, s in trig_defer:
                    act_chain.append(nc.scalar.activation(
                        vraw_t[s], src_t[s], Act.Sin, scale=1.0))
                if trig_defer:
                    _finalize(*defer0)

            # pin ACT table order: trig group then exp group, per layer.
            for a, b in zip(act_chain, act_chain[1:]):
                add_dep_helper(b.ins, a.ins, sync=False,
                               reason="act table order")
    nc.compile()
    return nc


_CACHE = {}


def _get_program(segs_key, segs, w):
    key = (segs_key, float(w))
    if key not in _CACHE:
        _CACHE[key] = _build_program(segs, w)
    return _CACHE[key]


def kernel(x, shared_weight, edge_src, edge_dst, act_ids):
    import concourse.mybir as mybir
    from concourse.bass_utils import run_bass_kernel_spmd

    w = float(np.asarray(shared_weight))
    assert w != 0.0
    a_pack, xin, perms, segs = _preprocess(
        np.asarray(x), w, np.asarray(edge_src), np.asarray(edge_dst),
        np.asarray(act_ids))

    segs_key = tuple(tuple(tuple(r) for r in lm) for lseg in segs for lm in lseg)
    nc = _get_program(segs_key, segs, w)

    dt = mybir.dt
    a_np = {"f32r": np.float32, "bf16": mybir.dt.np(dt.bfloat16),
            "fp8e4": mybir.dt.np(dt.float8e4)}[A_DTYPE]
    a_cast = a_pack.astype(a_np)

    in_maps = [
        {"xin": np.ascontiguousarray(xin[:, cid * BS:(cid + 1) * BS]),
         "amat": a_cast}
        for cid in range(N_CORES)
    ]
    res = run_bass_kernel_spmd(nc, in_maps, core_ids=list(range(N_CORES)))
    out_sorted = np.concatenate([res.results[cid]["out"]
                                 for cid in range(N_CORES)], axis=1)
    out = np.empty_like(out_sorted)
    out[perms[L - 1]] = out_sorted
    return out.astype(np.float32)
